# revision 1
# baseline (speedup 1.0000x reference)
"""Trainium2 Bass kernel for CharacterBERT CharCNN (char-CNN + highway + proj).

Self-contained: takes FULL inputs (as produced by the reference's
setup_inputs), shards the 4096 tokens data-parallel across 8 NeuronCores,
runs one SPMD Bass/Tile program per core, and gathers the full output.

Per-core pipeline (512 tokens):
  1. dma_gather (transpose) of a row-replicated bf16 char-embedding table
     -> x_rep [128, NIDX]: partitions = 8 replicated groups of 16 channels.
  2. 7 SBUF->SBUF "skew" DMAs build x_skew [120 rows, 26000 cols] where row
     16*dw+c at col (n,p) holds emb[ids[n, p+dw]][c]; rows 112..118 carry
     per-position filter-width mask indicators, row 119 is ones (bias row).
  3. Conv = single matmul per 128-filter chunk against a combined, width-
     padded weight matrix W_all [120, 2048] (mask rows inject -30000 at
     positions invalid for each filter's width; bias folded via ones row).
  4. Fused relu+masked-maxpool via DVE tensor_tensor_scan:
     state = max(state * m, y) with m=0 at each token's first position
     (reset-to-relu trick); token maxes extracted at pos 49 with strided
     copies -> t feature-major [2048, tokens].
  5. Two highway layers + projection as feature-major matmuls (weights are
     host-repacked so every DMA is contiguous); sigmoid/relu+bias on ACT
     straight from PSUM; combine on DVE in bf16.
  6. Output written feature-major [768, 512] fp32; transposed on host.
"""

import numpy as np
import ml_dtypes
from contextlib import ExitStack

import concourse.bass as bass
import concourse.mybir as mybir
import concourse.tile as tile
from concourse import bacc
from concourse.bass_utils import run_bass_kernel_spmd

BF16 = mybir.dt.bfloat16
F32 = mybir.dt.float32
I16 = mybir.dt.int16
AF = mybir.ActivationFunctionType

# problem geometry (hardcoded)
B, S, MAX_CHARS = 8, 512, 50
EMB = 16
VOCAB = 264
TOTAL_F = 2048
HIDDEN = 768
FILTERS = [(1, 32), (2, 32), (3, 64), (4, 128), (5, 256), (6, 512), (7, 1024)]
NCORES = 8

# per-core geometry
T = 512                  # real tokens per core
TPAD = 520               # padded tokens (multiple of 10 for 50-col grouping)
P50 = MAX_CHARS
COLS = TPAD * P50        # 26000
HCOLS = COLS // 2        # 13000 (260 tokens per half)
NIDX = 26112             # gather indices, %128 == 0, >= COLS + 6
KCONV = 120              # 112 patch rows + 7 mask rows + 1 bias row
NEG = -30000.0
SCAN_G = 1024
NKF = TOTAL_F // 128     # 16 filter chunks
NKH = TOTAL_F // 128     # 16 contraction chunks for highway/proj
NOH = 2 * TOTAL_F // 128  # 32 highway output chunks
NOP = HIDDEN // 128      # 6 proj output chunks
HALVES = ((0, 260), (260, 252))  # (token0, ntok) real tokens per half

_BF = ml_dtypes.bfloat16


def _bf(x):
    return np.asarray(x, dtype=np.float32).astype(_BF)


def build_program(ExitStackCls=ExitStack):
    """Build + compile the single-core SPMD Bass program. Returns nc."""
    nc = bacc.Bacc("TRN2", target_bir_lowering=False, debug=False)

    d_oh = nc.dram_tensor("oh", [VOCAB, NIDX], BF16, kind="ExternalInput").ap()
    d_embr = nc.dram_tensor("embr", [128, 384], BF16, kind="ExternalInput").ap()
    d_pat = nc.dram_tensor("pat", [8, COLS], BF16, kind="ExternalInput").ap()
    d_mmul = nc.dram_tensor("mmul", [128, SCAN_G + P50 - 1], BF16, kind="ExternalInput").ap()
    d_wall = nc.dram_tensor("wall", [KCONV, TOTAL_F], BF16, kind="ExternalInput").ap()
    d_hw0w = nc.dram_tensor("hw0w", [NOH, 128, TOTAL_F], BF16, kind="ExternalInput").ap()
    d_hw1w = nc.dram_tensor("hw1w", [NOH, 128, TOTAL_F], BF16, kind="ExternalInput").ap()
    d_prjw = nc.dram_tensor("prjw", [NOP, 128, TOTAL_F], BF16, kind="ExternalInput").ap()
    d_hwb = nc.dram_tensor("hwb", [128, 64], F32, kind="ExternalInput").ap()
    d_prjb = nc.dram_tensor("prjb", [128, NOP], F32, kind="ExternalInput").ap()
    d_out = nc.dram_tensor("out", [NOP, 128, T], F32, kind="ExternalOutput").ap()
    d_hw_w = [d_hw0w, d_hw1w]

    with tile.TileContext(nc) as tc, ExitStackCls() as ctx:
        const = ctx.enter_context(tc.tile_pool(name="const", bufs=1))
        oh_p = ctx.enter_context(tc.tile_pool(name="ohp", bufs=2))
        xrep_p = ctx.enter_context(tc.tile_pool(name="xrep", bufs=3))
        xskew_p = ctx.enter_context(tc.tile_pool(name="xskew", bufs=1))
        tmaj = ctx.enter_context(tc.tile_pool(name="tmaj", bufs=2))
        scano_p = ctx.enter_context(tc.tile_pool(name="scano", bufs=3))
        hww_p = ctx.enter_context(tc.tile_pool(name="hww", bufs=3))
        hwtmp = ctx.enter_context(tc.tile_pool(name="hwtmp", bufs=2))
        outp = ctx.enter_context(tc.tile_pool(name="outp", bufs=2))
        convps = ctx.enter_context(tc.tile_pool(name="convps", bufs=2, space="PSUM"))
        nlps = ctx.enter_context(tc.tile_pool(name="nlps", bufs=2, space="PSUM"))
        gps = ctx.enter_context(tc.tile_pool(name="gps", bufs=2, space="PSUM"))

        # ---- constants ----
        er_t = const.tile([128, 384], BF16)
        nc.sync.dma_start(er_t[:], d_embr[:])
        mm_t = const.tile([128, SCAN_G + P50 - 1], BF16)
        nc.sync.dma_start(mm_t[:], d_mmul[:])
        wall_t = const.tile([KCONV, TOTAL_F], BF16)
        nc.sync.dma_start(wall_t[:], d_wall[:])
        hwb_t = const.tile([128, 64], F32)
        nc.sync.dma_start(hwb_t[:], d_hwb[:])
        prjb_t = const.tile([128, NOP], F32)
        nc.sync.dma_start(prjb_t[:], d_prjb[:])

        # ---- embedding via onehot matmul, streamed in column chunks ----
        x_skew = xskew_p.tile([KCONV, COLS], BF16)
        nc.sync.dma_start(out=x_skew[112:120, :], in_=d_pat[:, :])
        GCH = 2048
        VCH = ((0, 128), (128, 128), (256, 8))
        for c0 in range(0, NIDX, GCH):
            n = min(GCH, NIDX - c0)
            ohts = []
            for vi, (v0, vn) in enumerate(VCH):
                oht = oh_p.tile([vn, GCH], BF16, tag=f"oh{vi}")
                nc.sync.dma_start(oht[:, :n], d_oh[v0:v0 + vn, c0:c0 + n])
                ohts.append(oht)
            xr = xrep_p.tile([128, GCH], BF16, tag="xr")
            for b in range(0, n, 512):
                ps = gps.tile([128, 512], F32, tag="gps")
                for vi, (v0, vn) in enumerate(VCH):
                    nc.tensor.matmul(
                        ps[:, :],
                        lhsT=er_t[:vn, 128 * vi:128 * (vi + 1)],
                        rhs=ohts[vi][:, b:b + 512],
                        start=(vi == 0), stop=(vi == 2),
                    )
                nc.scalar.copy(xr[:, b:b + 512], ps[:, :])
            # skew copies out of this chunk
            for g in range(7):
                lo = max(0, c0 - g)
                hi = min(c0 + n - g, COLS)
                if hi > lo:
                    nc.sync.dma_start(
                        out=x_skew[16 * g:16 * (g + 1), lo:hi],
                        in_=xr[16 * g:16 * (g + 1), lo - (c0 - g):hi - (c0 - g)])

        # ---- conv + scan-maxpool-relu -> tT [128, 16, TPAD] bf16 ----
        tT = tmaj.tile([128, NKF, TPAD], BF16, tag="t")
        for h in range(2):
            h0 = h * HCOLS
            for k in range(NKF):
                prev = None
                c0 = 0
                while c0 < HCOLS:
                    n = min(SCAN_G, HCOLS - c0)
                    ps = convps.tile([128, SCAN_G], F32)
                    for b in range(0, n, 512):
                        m = min(512, n - b)
                        nc.tensor.matmul(
                            ps[:, b:b + m],
                            lhsT=wall_t[:, 128 * k:128 * (k + 1)],
                            rhs=x_skew[:, h0 + c0 + b:h0 + c0 + b + m],
                            start=True, stop=True,
                        )
                    so = scano_p.tile([128, SCAN_G], BF16)
                    ph = c0 % P50
                    nc.vector.tensor_tensor_scan(
                        out=so[:, :n],
                        data0=mm_t[:, ph:ph + n],
                        data1=ps[:, :n],
                        initial=(0.0 if prev is None else prev),
                        op0=mybir.AluOpType.mult,
                        op1=mybir.AluOpType.max,
                    )
                    prev = so[:, n - 1:n]
                    first = (P50 - 1 - c0) % P50
                    if first < n:
                        cnt = (n - first + P50 - 1) // P50
                        tok0 = (h0 + c0 + first) // P50
                        src = (so[:, first:first + P50 * (cnt - 1) + 1:P50]
                               if cnt > 1 else so[:, first:first + 1])
                        nc.vector.tensor_copy(tT[:, k, tok0:tok0 + cnt], src)
                    c0 += n

        # ---- highway layers ----
        t_in = tT
        for layer in range(2):
            t_out = tmaj.tile([128, NKF, TPAD], BF16, tag="t")
            for j in range(NKH):
                w_nl = hww_p.tile([128, TOTAL_F], BF16)
                nc.sync.dma_start(w_nl[:], d_hw_w[layer][j, :, :])
                w_g = hww_p.tile([128, TOTAL_F], BF16)
                nc.sync.dma_start(w_g[:], d_hw_w[layer][j + 16, :, :])
                b_nl = hwb_t[:, layer * 32 + j:layer * 32 + j + 1]
                b_g = hwb_t[:, layer * 32 + 16 + j:layer * 32 + 16 + j + 1]
                for (t0, ntok) in HALVES:
                    ps_nl = nlps.tile([128, 512], F32, tag="hwps")
                    ps_g = gps.tile([128, 512], F32, tag="gps")
                    for kc in range(NKH):
                        nc.tensor.matmul(
                            ps_nl[:, :ntok],
                            lhsT=w_nl[:, 128 * kc:128 * (kc + 1)],
                            rhs=t_in[:, kc, t0:t0 + ntok],
                            start=(kc == 0), stop=(kc == NKH - 1),
                        )
                    for kc in range(NKH):
                        nc.tensor.matmul(
                            ps_g[:, :ntok],
                            lhsT=w_g[:, 128 * kc:128 * (kc + 1)],
                            rhs=t_in[:, kc, t0:t0 + ntok],
                            start=(kc == 0), stop=(kc == NKH - 1),
                        )
                    sg = hwtmp.tile([128, 512], BF16, tag="sg")
                    nc.scalar.activation(sg[:, :ntok], ps_g[:, :ntok], AF.Sigmoid, bias=b_g)
                    rl = hwtmp.tile([128, 512], BF16, tag="rl")
                    nc.scalar.activation(rl[:, :ntok], ps_nl[:, :ntok], AF.Relu, bias=b_nl)
                    dd = hwtmp.tile([128, 512], BF16, tag="dd")
                    nc.vector.tensor_sub(dd[:, :ntok], t_in[:, j, t0:t0 + ntok], rl[:, :ntok])
                    ee = hwtmp.tile([128, 512], BF16, tag="ee")
                    nc.vector.tensor_mul(ee[:, :ntok], sg[:, :ntok], dd[:, :ntok])
                    nc.vector.tensor_add(t_out[:, j, t0:t0 + ntok], ee[:, :ntok], rl[:, :ntok])
            t_in = t_out

        # ---- projection ----
        for o in range(NOP):
            w_p = hww_p.tile([128, TOTAL_F], BF16)
            nc.sync.dma_start(w_p[:], d_prjw[o, :, :])
            ot = outp.tile([128, T], F32)
            for (t0, ntok) in HALVES:
                ps = nlps.tile([128, 512], F32, tag="hwps")
                for kc in range(NKH):
                    nc.tensor.matmul(
                        ps[:, :ntok],
                        lhsT=w_p[:, 128 * kc:128 * (kc + 1)],
                        rhs=t_in[:, kc, t0:t0 + ntok],
                        start=(kc == 0), stop=(kc == NKH - 1),
                    )
                nc.scalar.activation(
                    ot[:, t0:t0 + ntok], ps[:, :ntok], AF.Identity,
                    bias=prjb_t[:, o:o + 1],
                )
            nc.sync.dma_start(out=d_out[o, :, :], in_=ot[:, :])

    nc.compile()
    return nc


# ---------------- host-side preparation ----------------

def prep_shared(char_emb, conv_ws, conv_bs, hw_ws, hw_bs, proj_w, proj_b):
    """Host repack of all parameters (shared across cores)."""
    out = {}
    # replicated bf16 embedding table, repacked for K-chunked lhsT [128, 384]
    emb_rep = np.zeros((384, 128), dtype=np.float32)
    emb_rep[:VOCAB] = np.tile(_bf(char_emb).astype(np.float32), (1, 8))
    embr = np.zeros((128, 384), dtype=_BF)
    for c in range(3):
        embr[:, 128 * c:128 * (c + 1)] = emb_rep[128 * c:128 * (c + 1), :].astype(_BF)
    out["embr"] = embr

    # pattern rows: j-indicator (rows 0..6) period 50, ones row (row 7)
    pat = np.zeros((8, COLS), dtype=_BF)
    pos = np.arange(COLS) % P50
    for j in range(7):
        pat[j] = (pos >= P50 - j).astype(_BF)
    pat[7] = 1.0
    out["pat"] = pat

    # scan multiplier mask, periodic phase tile
    mpos = np.arange(SCAN_G + P50 - 1) % P50
    out["mmul"] = np.tile((mpos != 0).astype(_BF), (128, 1))

    # combined conv weight [120, 2048]
    wall = np.zeros((KCONV, TOTAL_F), dtype=np.float32)
    fbase = 0
    for (w, nf), cw, cb in zip(FILTERS, conv_ws, conv_bs):
        cw = np.asarray(cw, np.float32)  # [nf, 16, w]
        for dw in range(w):
            wall[16 * dw:16 * (dw + 1), fbase:fbase + nf] = cw[:, :, dw].T
        wall[112 + (w - 1), fbase:fbase + nf] = NEG if w > 1 else 0.0
        wall[119, fbase:fbase + nf] = np.asarray(cb, np.float32)
        fbase += nf
    out["wall"] = wall.astype(_BF)

    # highway / proj weights repacked [o, partition(k-in-chunk), kc*128+m]
    def repack(wm, no):
        wm = np.asarray(wm, np.float32)
        kk = wm.shape[0] // 128
        return (_bf(wm).reshape(kk, 128, no, 128)
                .transpose(2, 1, 0, 3).reshape(no, 128, kk * 128))

    out["hw0w"] = repack(hw_ws[0], NOH)
    out["hw1w"] = repack(hw_ws[1], NOH)
    out["prjw"] = repack(proj_w, NOP)

    # biases: hwb [128, 64]: col layout layer*32 + which*16 + j
    hwb = np.zeros((128, 64), dtype=np.float32)
    for layer in range(2):
        hb = np.asarray(hw_bs[layer], np.float32)
        for j in range(16):
            hwb[:, layer * 32 + j] = hb[128 * j:128 * (j + 1)]
            hwb[:, layer * 32 + 16 + j] = hb[TOTAL_F + 128 * j:TOTAL_F + 128 * (j + 1)]
    out["hwb"] = hwb
    out["prjb"] = np.asarray(proj_b, np.float32).reshape(NOP, 128).T.copy()
    return out


def prep_oh(ids_core):
    """ids_core [T, 50] int -> onehot [VOCAB, NIDX] bf16 (pad cols all-zero)."""
    flat = ids_core.reshape(-1).astype(np.int64)
    oh = np.zeros((VOCAB, NIDX), dtype=_BF)
    oh[flat, np.arange(T * P50)] = 1.0
    return oh


_CACHED_NC = None


def _get_nc():
    global _CACHED_NC
    if _CACHED_NC is None:
        _CACHED_NC = build_program()
    return _CACHED_NC


def make_in_maps(inputs):
    ii = {k: np.asarray(v) for k, v in inputs.items()}
    conv_ws = [ii[f"conv_w{i}"] for i in range(7)]
    conv_bs = [ii[f"conv_b{i}"] for i in range(7)]
    shared = prep_shared(
        ii["char_emb"], conv_ws, conv_bs,
        [ii["hw_w0"], ii["hw_w1"]], [ii["hw_b0"], ii["hw_b1"]],
        ii["proj_w"], ii["proj_b"],
    )
    ids = ii["input_ids"].reshape(-1, MAX_CHARS)  # [4096, 50]
    in_maps = []
    for c in range(NCORES):
        m = dict(shared)
        m["oh"] = prep_oh(ids[c * T:(c + 1) * T])
        in_maps.append(m)
    return in_maps


def run(inputs, trace=False, **kw):
    """Run on 8 cores; returns (full_output, BassKernelResults)."""
    in_maps = make_in_maps(inputs)
    res = run_bass_kernel_spmd(_get_nc(), in_maps, list(range(NCORES)),
                               trace=trace, **kw)
    outs = []
    for c in range(NCORES):
        o = np.asarray(res.results[c]["out"])  # [6, 128, T] fp32
        outs.append(o.reshape(HIDDEN, T).T)   # [T, 768]
    full = np.stack(outs, axis=0).reshape(B, S, HIDDEN).astype(np.float32)
    return full, res


def kernel(**inputs):
    return run(inputs)[0]


if __name__ == "__main__":
    # smoke: build only
    build_program()
    print("build ok")

